# revision 1
# baseline (speedup 1.0000x reference)
"""Grouped MLP (MoE) Trainium2 kernel: 8 experts x 1024 tokens, H=2048, I=5632, GLU.

Expert-parallel sharding: core i handles expert i's full MLP (zero cross-core
communication). Per core:
    fc1T = w1_e.T @ x_e.T        (bf16 matmuls, PSUM fp32 accum over H)
    inter = silu(a) * b          (GLU on ACT+DVE straight out of PSUM)
    out_e = inter.T @ w2_e       (bf16, pair-wise PSUM accum + SBUF fp32 accumulator)

All matmuls run in bf16 (measured 216ns/MM sustained at N=512 vs 227ns for
f32r — FWL halves the weight-load; ~4e-3 max rel err end to end, well under
the 2e-2 gate). Weights are DMA'd as f32, cast to bf16 on the otherwise-idle
GpSimd engine. x streams on the sync ring at full rate; the steady-state
weight stream stays on the sync ring only (concurrent rings thrash SBUF
banks); output is written per token-tile so the tail overlaps compute.
"""

import sys

sys.path.insert(0, "/opt/trn_rl_repo")

import numpy as np

E, T, H, I = 8, 1024, 2048, 5632
TWO_I = 2 * I
P = 128
KO = H // P        # 16  k-subtiles for GEMM1
NJ = I // P        # 44  column tiles of I (= k-tiles for GEMM2)
NT = T // P        # 8   token tiles
NH = H // 512      # 4   output column tiles
N_CORES = 8

_RUNNER = None


def _build_program(reps: int = 1):
    import concourse.bacc as bacc
    import concourse.mybir as mybir
    import concourse.tile as tile
    from concourse.masks import make_identity

    f32 = mybir.dt.float32
    bf16 = mybir.dt.bfloat16

    nc = bacc.Bacc("TRN2", target_bir_lowering=False, debug=False,
                   num_devices=N_CORES)
    x = nc.dram_tensor("x", [T, H], f32, kind="ExternalInput").ap()
    w1 = nc.dram_tensor("w1", [H, TWO_I], f32, kind="ExternalInput").ap()
    w2 = nc.dram_tensor("w2", [I, H], f32, kind="ExternalInput").ap()
    out = nc.dram_tensor("out", [T, H], f32, kind="ExternalOutput").ap()

    # K-on-partitions views (partition index is the inner row index)
    w1v = w1.rearrange("(ko p) m -> p ko m", p=P)        # [128,16,11264] f32
    w2v = w2.rearrange("(j p) h -> p j h", p=P)          # [128,44,2048]  f32
    x_t = x.rearrange("(to p) h -> p to h", p=P)         # [128,8,2048]
    out_t = out.rearrange("(to p) h -> p to h", p=P)     # [128,8,2048]

    # Two HWDGE rings; ring 0 = sync (bulk stream), ring 1 = scalar (prologue
    # wa0 + tail outs only — two concurrent weight streams thrash SBUF banks).
    def dma(ring, dst, src):
        (nc.sync if ring == 0 else nc.scalar).dma_start(dst, src)

    with tile.TileContext(nc) as tc:
        with (
            tc.tile_pool(name="const", bufs=1) as const,
            tc.tile_pool(name="xT", bufs=1) as xT_pool,
            tc.tile_pool(name="w1p", bufs=4) as w1p,
            tc.tile_pool(name="w2p", bufs=3) as w2p,
            tc.tile_pool(name="stg1", bufs=2) as stg1,
            tc.tile_pool(name="stg2", bufs=2) as stg2,
            tc.tile_pool(name="interp", bufs=3) as interp,
            tc.tile_pool(name="tmpp", bufs=2) as tmpp,
            tc.tile_pool(name="oacc", bufs=1) as oacc_pool,
        ):
            ident = const.tile([P, P], f32)
            make_identity(nc, ident)
            zeros = const.tile([P, 512], f32)
            nc.vector.memset(zeros[:], 0.0)

            xT = xT_pool.tile([P, KO, T], bf16)            # 32 KB/partition
            out_acc = oacc_pool.tile([P, NT, H], f32)      # 64 KB/partition

            def load_w1_pair(c0, ring):
                # one 256-col slice (1KB contiguous chunks — 2x the HBM
                # efficiency of per-128-col loads), cast to bf16 on GpSimd.
                # Cast in two ko-halves so the first GEMM1 MMs can start
                # before the whole slice is converted.
                stg = stg1.tile([P, KO, 2 * P], f32, tag="s1", name=f"s1_{c0}")
                dma(ring, stg[:], w1v[:, :, c0 * P:(c0 + 2) * P])
                wt = w1p.tile([P, KO, 2 * P], bf16, tag="w1t", name=f"w1_{c0}")
                half = KO // 2
                nc.gpsimd.tensor_copy(wt[:, :half], stg[:, :half])
                nc.gpsimd.tensor_copy(wt[:, half:], stg[:, half:])
                return wt

            def load_w2(j, ring):
                stg = stg2.tile([P, H], f32, tag="s2", name=f"s2_{j}")
                dma(ring, stg[:], w2v[:, j])
                wt = w2p.tile([P, H], bf16, tag="w2t", name=f"w2_{j}")
                nc.gpsimd.tensor_copy(wt[:], stg[:])
                return wt

            for _rep in range(reps):
                # ---- prologue: x streams on the sync ring at full rate;
                # only pair-0's 'a' w1 slice rides the scalar ring (strided w1
                # reads are HBM-inefficient — more concurrent prologue weight
                # traffic starves the x stream). Everything else queues on
                # sync right behind x, in consumption order.
                # pair-0's w1 loads FIRST (before x) — everything on the sync
                # ring in consumption-deadline order; the scalar ring carries
                # only the tail output DMAs (its dma_start issues would block
                # the ACT engine's transpose copies for ~10us otherwise).
                wa_p0 = load_w1_pair(0, 0)
                wb_p0 = load_w1_pair(NJ, 0)
                for to in range(NT):
                    dma(0, out_acc[:, to], x_t[:, to])
                w2_first = [load_w2(0, 0), load_w2(1, 0)]
                wa_p1 = load_w1_pair(2, 0)
                wb_p1 = load_w1_pair(NJ + 2, 0)

                with (
                    tc.tile_pool(name="psum1", bufs=4, space="PSUM") as psum1,
                    tc.tile_pool(name="psum2", bufs=4, space="PSUM") as psum2,
                ):
                    def transpose_tiles(to_lo, to_hi):
                        # Transposes borrow psum2's banks (same tag/shape as
                        # the GEMM2 po tiles). Copies avoid DVE tensor_copy
                        # (its 2-port mode serializes against the GpSimd
                        # weight casts on the shared SBUF port pair): ACT copy
                        # and DVE tensor_tensor(+0) never contend.
                        for to in range(to_lo, to_hi):
                            stage = out_acc[:, to]
                            for ko in range(KO):
                                pst = psum2.tile([P, 512], f32, tag="po",
                                                 name=f"pt{to}_{ko}")
                                nc.tensor.transpose(
                                    pst[:, 0:P],
                                    stage[:, ko * P:(ko + 1) * P], ident[:]
                                )
                                dst = xT[:, ko, to * P:(to + 1) * P]
                                if ko % 2 == 0:
                                    nc.vector.tensor_tensor(
                                        dst, pst[:, 0:P], zeros[:, 0:P],
                                        mybir.AluOpType.add
                                    )
                                else:
                                    nc.scalar.copy(dst, pst[:, 0:P])
                    def gemm1_half(wtile, jj, th, pout):
                        # one (a or b, th) accumulation: 16 MMs of N=512
                        for ko in range(KO):
                            nc.tensor.matmul(
                                pout[:], wtile[:, ko, jj * P:(jj + 1) * P],
                                xT[:, ko, th * 512:(th + 1) * 512],
                                start=(ko == 0), stop=(ko == KO - 1),
                            )

                    def glu(it, th, pa, pb):
                        # silu on ACT (f32 tmp), mult+cast-to-bf16 on DVE
                        tmp = tmpp.tile([P, 512], f32, tag="tmp")
                        nc.scalar.activation(
                            tmp[:], pa[:], mybir.ActivationFunctionType.Silu
                        )
                        sl = it[:, th * 512:(th + 1) * 512]
                        nc.vector.tensor_tensor(
                            sl, tmp[:], pb[:], mybir.AluOpType.mult
                        )

                    inter_tiles = [None, None]
                    w2_tiles = [None, None]
                    for jp in range(NJ // 2):
                        if jp == 0:
                            wa_pair, wb_pair = wa_p0, wb_p0
                        elif jp == 1:
                            wa_pair, wb_pair = wa_p1, wb_p1
                        else:
                            wa_pair = load_w1_pair(2 * jp, 0)
                            wb_pair = load_w1_pair(NJ + 2 * jp, 0)
                        for jj in range(2):
                            j = 2 * jp + jj
                            it = interp.tile([P, T], bf16, tag="it")
                            if j == 0:
                                # th-split: GEMM1 starts after only 4 x-tiles
                                # are transposed; tiles 4-7 transpose during
                                # the th0 MMs.
                                transpose_tiles(0, 4)
                                pa = psum1.tile([P, 512], f32, tag="pg1")
                                pb = psum1.tile([P, 512], f32, tag="pg1")
                                gemm1_half(wa_pair, jj, 0, pa)
                                gemm1_half(wb_pair, jj, 0, pb)
                                glu(it, 0, pa, pb)
                                transpose_tiles(4, 8)
                                pa = psum1.tile([P, 512], f32, tag="pg1")
                                pb = psum1.tile([P, 512], f32, tag="pg1")
                                gemm1_half(wa_pair, jj, 1, pa)
                                gemm1_half(wb_pair, jj, 1, pb)
                                glu(it, 1, pa, pb)
                            else:
                                for th in range(2):
                                    pa = psum1.tile([P, 512], f32, tag="pg1")
                                    pb = psum1.tile([P, 512], f32, tag="pg1")
                                    gemm1_half(wa_pair, jj, th, pa)
                                    gemm1_half(wb_pair, jj, th, pb)
                                    glu(it, th, pa, pb)
                            inter_tiles[jj] = it
                            w2_tiles[jj] = (w2_first[j] if j < 2
                                            else load_w2(j, 0))

                        # GEMM2 partial for this pair, accumulate into out_acc
                        for t in range(NT):
                            for h in range(NH):
                                po = psum2.tile([P, 512], f32, tag="po")
                                nc.tensor.matmul(
                                    po[:],
                                    inter_tiles[0][:, t * P:(t + 1) * P],
                                    w2_tiles[0][:, h * 512:(h + 1) * 512],
                                    start=True, stop=False,
                                )
                                nc.tensor.matmul(
                                    po[:],
                                    inter_tiles[1][:, t * P:(t + 1) * P],
                                    w2_tiles[1][:, h * 512:(h + 1) * 512],
                                    start=False, stop=True,
                                )
                                dst = out_acc[:, t, h * 512:(h + 1) * 512]
                                if jp == 0:
                                    # po+0: copy without DVE's 2-port mode
                                    # (which would block GpSimd casts)
                                    nc.vector.tensor_tensor(
                                        dst, po[:], zeros[:],
                                        mybir.AluOpType.add
                                    )
                                else:
                                    nc.vector.tensor_tensor(
                                        dst, po[:], dst, mybir.AluOpType.add
                                    )

                # ---- Output: per token-tile on the scalar ring so the tail
                # overlaps the last pair's compute and weight stream ----
                for t in range(NT):
                    dma(1, out_t[:, t], out_acc[:, t])

    nc.compile()
    return nc


def _build_runner(nc):
    import jax
    from jax.sharding import Mesh, PartitionSpec
    from jax.experimental.shard_map import shard_map
    import concourse.mybir as mybir
    from concourse.bass2jax import (
        _bass_exec_p, install_neuronx_cc_hook, partition_id_tensor,
    )

    install_neuronx_cc_hook()
    partition_name = (
        nc.partition_id_tensor.name if nc.partition_id_tensor else None
    )
    in_names, out_names, out_avals, zero_shapes = [], [], [], []
    for alloc in nc.m.functions[0].allocations:
        if not isinstance(alloc, mybir.MemoryLocationSet):
            continue
        name = alloc.memorylocations[0].name
        if alloc.kind == "ExternalInput":
            if name != partition_name:
                in_names.append(name)
        elif alloc.kind == "ExternalOutput":
            out_names.append(name)
            shape = tuple(alloc.tensor_shape)
            dtype = mybir.dt.np(alloc.dtype)
            out_avals.append(jax.core.ShapedArray(shape, dtype))
            zero_shapes.append((shape, dtype))
    n_params = len(in_names)
    n_outs = len(out_avals)
    all_in_names = list(in_names) + list(out_names)
    if partition_name is not None:
        all_in_names.append(partition_name)

    def _body(*args):
        operands = list(args)
        if partition_name is not None:
            operands.append(partition_id_tensor())
        outs = _bass_exec_p.bind(
            *operands,
            out_avals=tuple(out_avals),
            in_names=tuple(all_in_names),
            out_names=tuple(out_names),
            lowering_input_output_aliases=(),
            sim_require_finite=True,
            sim_require_nnan=True,
            nc=nc,
        )
        return tuple(outs)

    devices = jax.devices()[:N_CORES]
    mesh = Mesh(np.asarray(devices), ("core",))
    in_specs = (PartitionSpec("core"),) * (n_params + n_outs)
    out_specs = (PartitionSpec("core"),) * n_outs
    sharded = jax.jit(
        shard_map(_body, mesh=mesh, in_specs=in_specs, out_specs=out_specs,
                  check_rep=False),
        keep_unused=True,
    )

    def run(in_maps):
        concat_in = [
            np.concatenate([np.asarray(m[n]) for m in in_maps], axis=0)
            for n in in_names
        ]
        concat_zeros = [
            np.zeros((N_CORES * s[0], *s[1:]), dt) for s, dt in zero_shapes
        ]
        out_arrs = sharded(*concat_in, *concat_zeros)
        return [
            {n: np.asarray(out_arrs[i]).reshape(N_CORES, *out_avals[i].shape)[c]
             for i, n in enumerate(out_names)}
            for c in range(N_CORES)
        ]

    run.sharded = sharded
    run.in_names = in_names
    run.zero_shapes = zero_shapes
    return run


def _get_runner(reps: int = 1):
    global _RUNNER
    if _RUNNER is None or _RUNNER[1] != reps:
        nc = _build_program(reps)
        _RUNNER = (_build_runner(nc), reps)
    return _RUNNER[0]


def kernel(permuted_hidden_states, w1, w2, tokens_per_expert):
    run = _get_runner()
    phs = np.ascontiguousarray(np.asarray(permuted_hidden_states, dtype=np.float32))
    w1 = np.asarray(w1, dtype=np.float32)
    w2 = np.asarray(w2, dtype=np.float32)
    in_maps = [
        {
            "x": phs[e * T:(e + 1) * T],
            "w1": np.ascontiguousarray(w1[e]),
            "w2": np.ascontiguousarray(w2[e]),
        }
        for e in range(E)
    ]
    res = run(in_maps)
    return np.concatenate([res[e]["out"] for e in range(E)], axis=0)



# revision 2
# speedup vs baseline: 1.0174x; 1.0174x over previous
"""Grouped MLP (MoE) Trainium2 kernel: 8 experts x 1024 tokens, H=2048, I=5632, GLU.

Expert-parallel sharding: core i handles expert i's full MLP (zero cross-core
communication). Per core:
    fc1T = w1_e.T @ x_e.T        (bf16 matmuls, PSUM fp32 accum over H)
    inter = silu(a) * b          (GLU on ACT+DVE straight out of PSUM)
    out_e = inter.T @ w2_e       (bf16, 4-deep PSUM accum + SBUF fp32 accumulator)

All matmuls run in bf16 (216ns/MM sustained at N=512; ~4e-3 max rel err end to
end, well under the 2e-2 gate). Weights are cast f32->bf16 INSIDE the DMA
(SWDGE cast-dma, measured 385 GB/s read-side for the whole 138MB/core weight
stream — faster than the HWDGE f32 copy and zero engine time, vs ~920us of
GpSimd CAST work in the previous version, which sat within 3% of the critical
path and stalled the PE mid-kernel). x + outputs ride the HWDGE sync ring,
which is independent of the SWDGE path at the issue level. GEMM2 accumulates
4 j-slices per PSUM bank before draining (halves DVE drain traffic); round-0
drains are plain copies on the otherwise-idle ACT engine. No DVE tensor_copy
anywhere — copy-class DVE ops grab the shared SBUF port pair and would starve
SWDGE descriptor generation.
"""

import sys

sys.path.insert(0, "/opt/trn_rl_repo")

import numpy as np

E, T, H, I = 8, 1024, 2048, 5632
TWO_I = 2 * I
P = 128
KO = H // P        # 16  k-subtiles for GEMM1
NJ = I // P        # 44  column tiles of I (= k-tiles for GEMM2)
NT = T // P        # 8   token tiles
NH = H // 512      # 4   output column tiles
G = 4              # j-slices accumulated per PSUM bank in GEMM2 (NJ/G rounds)
NG = NJ // G       # 11  GEMM2 accumulation rounds
N_CORES = 8

_RUNNER = None


def _build_program(reps: int = 1):
    import concourse.bacc as bacc
    import concourse.mybir as mybir
    import concourse.tile as tile
    from concourse.masks import make_identity

    f32 = mybir.dt.float32
    bf16 = mybir.dt.bfloat16

    nc = bacc.Bacc("TRN2", target_bir_lowering=False, debug=False,
                   num_devices=N_CORES)
    x = nc.dram_tensor("x", [T, H], f32, kind="ExternalInput").ap()
    w1 = nc.dram_tensor("w1", [H, TWO_I], f32, kind="ExternalInput").ap()
    w2 = nc.dram_tensor("w2", [I, H], f32, kind="ExternalInput").ap()
    out = nc.dram_tensor("out", [T, H], f32, kind="ExternalOutput").ap()

    # K-on-partitions views (partition index is the inner row index)
    w1v = w1.rearrange("(ko p) m -> p ko m", p=P)        # [128,16,11264] f32
    w2v = w2.rearrange("(j p) h -> p j h", p=P)          # [128,44,2048]  f32
    x_t = x.rearrange("(to p) h -> p to h", p=P)         # [128,8,2048]
    out_t = out.rearrange("(to p) h -> p to h", p=P)     # [128,8,2048]

    with tile.TileContext(nc) as tc:
        with (
            tc.tile_pool(name="const", bufs=1) as const,
            tc.tile_pool(name="xT", bufs=1) as xT_pool,
            tc.tile_pool(name="w1p", bufs=6) as w1p,
            tc.tile_pool(name="w2p", bufs=6) as w2p,
            tc.tile_pool(name="interp", bufs=6) as interp,
            tc.tile_pool(name="tmpp", bufs=2) as tmpp,
            tc.tile_pool(name="oacc", bufs=1) as oacc_pool,
        ):
            ident = const.tile([P, P], f32)
            make_identity(nc, ident)
            zeros = const.tile([P, 512], f32)
            nc.vector.memset(zeros[:], 0.0)

            xT = xT_pool.tile([P, KO, T], bf16)            # 32 KB/partition
            out_acc = oacc_pool.tile([P, NT, H], f32)      # 64 KB/partition

            def load_w1_pair(c0):
                # one 256-col slice of w1, f32 HBM -> bf16 SBUF via SWDGE
                # cast-dma (1KB contiguous chunks on the read side).
                wt = w1p.tile([P, KO, 2 * P], bf16, tag="w1t", name=f"w1_{c0}")
                nc.gpsimd.dma_start(wt[:], w1v[:, :, c0 * P:(c0 + 2) * P])
                return wt

            def load_w2(j):
                wt = w2p.tile([P, H], bf16, tag="w2t", name=f"w2_{j}")
                nc.gpsimd.dma_start(wt[:], w2v[:, j])
                return wt

            for _rep in range(reps):
                # ---- prologue: x first on the HWDGE sync ring (weights ride
                # the independent SWDGE path and overlap it).
                for to in range(NT):
                    nc.sync.dma_start(out_acc[:, to], x_t[:, to])

                with (
                    tc.tile_pool(name="psum1", bufs=4, space="PSUM") as psum1,
                    tc.tile_pool(name="psum2", bufs=4, space="PSUM") as psum2,
                ):
                    def transpose_tiles(to_lo, to_hi):
                        # Transposes borrow psum2's banks (same tag/shape as
                        # the GEMM2 po tiles). PSUM evacuation alternates ACT
                        # copy / DVE tensor_tensor(+0) — both 1-port ops.
                        for to in range(to_lo, to_hi):
                            stage = out_acc[:, to]
                            for ko in range(KO):
                                pst = psum2.tile([P, 512], f32, tag="po",
                                                 name=f"pt{to}_{ko}")
                                nc.tensor.transpose(
                                    pst[:, 0:P],
                                    stage[:, ko * P:(ko + 1) * P], ident[:]
                                )
                                dst = xT[:, ko, to * P:(to + 1) * P]
                                if ko % 2 == 0:
                                    nc.vector.tensor_tensor(
                                        dst, pst[:, 0:P], zeros[:, 0:P],
                                        mybir.AluOpType.add
                                    )
                                else:
                                    nc.scalar.copy(dst, pst[:, 0:P])

                    def gemm1_half(wtile, jj2, th, pout):
                        # one (a or b, th) accumulation: 16 MMs of N=512
                        for ko in range(KO):
                            nc.tensor.matmul(
                                pout[:], wtile[:, ko, jj2 * P:(jj2 + 1) * P],
                                xT[:, ko, th * 512:(th + 1) * 512],
                                start=(ko == 0), stop=(ko == KO - 1),
                            )

                    def glu(it, th, pa, pb):
                        # silu on ACT (f32 tmp), mult+cast-to-bf16 on DVE
                        tmp = tmpp.tile([P, 512], f32, tag="tmp")
                        nc.scalar.activation(
                            tmp[:], pa[:], mybir.ActivationFunctionType.Silu
                        )
                        sl = it[:, th * 512:(th + 1) * 512]
                        nc.vector.tensor_tensor(
                            sl, tmp[:], pb[:], mybir.AluOpType.mult
                        )

                    inter_tiles = [None] * G
                    w2_tiles = [None] * G
                    for g in range(NG):
                        wa0 = load_w1_pair(G * g)
                        wb0 = load_w1_pair(NJ + G * g)
                        wa1 = load_w1_pair(G * g + 2)
                        wb1 = load_w1_pair(NJ + G * g + 2)
                        for jj in range(G):
                            j = G * g + jj
                            wa = (wa0, wa1)[jj // 2]
                            wb = (wb0, wb1)[jj // 2]
                            jj2 = jj % 2
                            it = interp.tile([P, T], bf16, tag="it")
                            if j == 0:
                                # th-split: GEMM1 starts after only 4 x-tiles
                                # are transposed; tiles 4-7 transpose during
                                # the th0 MMs.
                                transpose_tiles(0, 4)
                                pa = psum1.tile([P, 512], f32, tag="pg1")
                                pb = psum1.tile([P, 512], f32, tag="pg1")
                                gemm1_half(wa, jj2, 0, pa)
                                gemm1_half(wb, jj2, 0, pb)
                                glu(it, 0, pa, pb)
                                transpose_tiles(4, 8)
                                pa = psum1.tile([P, 512], f32, tag="pg1")
                                pb = psum1.tile([P, 512], f32, tag="pg1")
                                gemm1_half(wa, jj2, 1, pa)
                                gemm1_half(wb, jj2, 1, pb)
                                glu(it, 1, pa, pb)
                            else:
                                for th in range(2):
                                    pa = psum1.tile([P, 512], f32, tag="pg1")
                                    pb = psum1.tile([P, 512], f32, tag="pg1")
                                    gemm1_half(wa, jj2, th, pa)
                                    gemm1_half(wb, jj2, th, pb)
                                    glu(it, th, pa, pb)
                            inter_tiles[jj] = it
                            w2_tiles[jj] = load_w2(j)

                        # GEMM2 partial for this group: G MMs accumulate per
                        # PSUM bank, then one drain into out_acc.
                        for t in range(NT):
                            for h in range(NH):
                                po = psum2.tile([P, 512], f32, tag="po")
                                for jj in range(G):
                                    nc.tensor.matmul(
                                        po[:],
                                        inter_tiles[jj][:, t * P:(t + 1) * P],
                                        w2_tiles[jj][:, h * 512:(h + 1) * 512],
                                        start=(jj == 0), stop=(jj == G - 1),
                                    )
                                dst = out_acc[:, t, h * 512:(h + 1) * 512]
                                if g == 0:
                                    # overwrite: plain copy on idle ACT
                                    nc.scalar.copy(dst, po[:])
                                else:
                                    nc.vector.tensor_tensor(
                                        dst, po[:], dst, mybir.AluOpType.add
                                    )
                            if g == NG - 1:
                                # final round: stream this token tile out now
                                nc.sync.dma_start(out_t[:, t], out_acc[:, t])

    nc.compile()
    return nc


def _build_runner(nc):
    import jax
    from jax.sharding import Mesh, PartitionSpec
    from jax.experimental.shard_map import shard_map
    import concourse.mybir as mybir
    from concourse.bass2jax import (
        _bass_exec_p, install_neuronx_cc_hook, partition_id_tensor,
    )

    install_neuronx_cc_hook()
    partition_name = (
        nc.partition_id_tensor.name if nc.partition_id_tensor else None
    )
    in_names, out_names, out_avals, zero_shapes = [], [], [], []
    for alloc in nc.m.functions[0].allocations:
        if not isinstance(alloc, mybir.MemoryLocationSet):
            continue
        name = alloc.memorylocations[0].name
        if alloc.kind == "ExternalInput":
            if name != partition_name:
                in_names.append(name)
        elif alloc.kind == "ExternalOutput":
            out_names.append(name)
            shape = tuple(alloc.tensor_shape)
            dtype = mybir.dt.np(alloc.dtype)
            out_avals.append(jax.core.ShapedArray(shape, dtype))
            zero_shapes.append((shape, dtype))
    n_params = len(in_names)
    n_outs = len(out_avals)
    all_in_names = list(in_names) + list(out_names)
    if partition_name is not None:
        all_in_names.append(partition_name)

    def _body(*args):
        operands = list(args)
        if partition_name is not None:
            operands.append(partition_id_tensor())
        outs = _bass_exec_p.bind(
            *operands,
            out_avals=tuple(out_avals),
            in_names=tuple(all_in_names),
            out_names=tuple(out_names),
            lowering_input_output_aliases=(),
            sim_require_finite=True,
            sim_require_nnan=True,
            nc=nc,
        )
        return tuple(outs)

    devices = jax.devices()[:N_CORES]
    mesh = Mesh(np.asarray(devices), ("core",))
    in_specs = (PartitionSpec("core"),) * (n_params + n_outs)
    out_specs = (PartitionSpec("core"),) * n_outs
    sharded = jax.jit(
        shard_map(_body, mesh=mesh, in_specs=in_specs, out_specs=out_specs,
                  check_rep=False),
        keep_unused=True,
    )

    def run(in_maps):
        concat_in = [
            np.concatenate([np.asarray(m[n]) for m in in_maps], axis=0)
            for n in in_names
        ]
        concat_zeros = [
            np.zeros((N_CORES * s[0], *s[1:]), dt) for s, dt in zero_shapes
        ]
        out_arrs = sharded(*concat_in, *concat_zeros)
        return [
            {n: np.asarray(out_arrs[i]).reshape(N_CORES, *out_avals[i].shape)[c]
             for i, n in enumerate(out_names)}
            for c in range(N_CORES)
        ]

    run.sharded = sharded
    run.in_names = in_names
    run.zero_shapes = zero_shapes
    return run


def _get_runner(reps: int = 1):
    global _RUNNER
    if _RUNNER is None or _RUNNER[1] != reps:
        nc = _build_program(reps)
        _RUNNER = (_build_runner(nc), reps)
    return _RUNNER[0]


def kernel(permuted_hidden_states, w1, w2, tokens_per_expert):
    run = _get_runner()
    phs = np.ascontiguousarray(np.asarray(permuted_hidden_states, dtype=np.float32))
    w1 = np.asarray(w1, dtype=np.float32)
    w2 = np.asarray(w2, dtype=np.float32)
    in_maps = [
        {
            "x": phs[e * T:(e + 1) * T],
            "w1": np.ascontiguousarray(w1[e]),
            "w2": np.ascontiguousarray(w2[e]),
        }
        for e in range(E)
    ]
    res = run(in_maps)
    return np.concatenate([res[e]["out"] for e in range(E)], axis=0)
